# revision 3
# baseline (speedup 1.0000x reference)
"""DeepSeek-MoE layer Trainium2 Bass kernel, v2: sparse expert-sorted routing.

Strategy (vs the dense data-parallel baseline):

  - Tokens stay data-parallel for the shared expert + router: core c owns the
    512-token chunk c (one batch row per 2 chunks, so t_emb is per-chunk
    constant). Router runs on device in f32, identical math to the reference,
    and emits the gate table `comb` (pre-scaled by 1/(N_SHARED+K)).
  - Routed experts are evaluated SPARSELY: only the top-2 (token, expert)
    pairs are computed. The host replicates the router math in numpy (same
    f32 inputs -> same top-2 sets) purely to build the data layout: all 8192
    selected pairs are sorted by expert, each expert group padded to a
    multiple of 384 slots, and the resulting expert-pure segments of 384
    slots are dealt round-robin free to the 8 cores (expert-parallel over
    segments). Each core receives, via its in_map, the gathered token
    activations for its segments (xgT) and the expert weight stacks those
    segments use (Wstk1/Wstk2) - this is the "all-to-all tokens by expert"
    of the sharding hint, performed during the host-side shard step.
  - Each core computes y = W2e @ gelu(W1e @ x_slot) for its slots (ungated),
    plus the shared expert for its own chunk. The host unshard step applies
    the DEVICE-computed gates from `comb` and scatter-adds y back to tokens.
  - All big matmuls run in bf16 (full-rate PE, 1 cyc/row vs 4 for f32;
    halves weight DMA). The router stays f32 so the device/host top-2 sets
    agree to f32 rounding.

All activations live transposed ([feature, token]) so both matmul layers
consume them directly as PE operands; layouts are prepared host-side.
"""

import os
import numpy as np
from contextlib import ExitStack

import concourse.bacc as bacc
import concourse.tile as tile
from concourse import mybir
from concourse.alu_op_type import AluOpType
from concourse.masks import make_identity
from concourse.bass_utils import run_bass_kernel_spmd

f32 = mybir.dt.float32
bf16 = mybir.dt.bfloat16

# problem shapes (hardcoded per contract)
D, HS, HE, E, TOPK = 1024, 2048, 1024, 8, 2
B, T = 4, 1024
NCORES = 8
NTOK = (B * T) // NCORES  # 512 tokens per core
P = 128
NT = NTOK // P  # 4 token tiles
KD = D // P     # 8 contraction chunks over D
KH = HE // P    # 8 contraction chunks over HE
KS = HS // P    # 16 contraction chunks over HS
ST = 3          # slot tiles per routed segment
SEG_SLOTS = ST * P  # 384 slots per segment (expert-pure)
OUT_SCALE = 1.0 / 3.0  # 1/(N_SHARED + TOPK)

# CoreSim doesn't implement Silu/Gelu; compat mode composes them from
# Sigmoid (silu exactly; gelu via the 1.702-sigmoid approx - wiring check
# only, hardware always runs the real thing).
ACT_COMPAT = os.environ.get("MOE_ACT_COMPAT", "0") == "1"
PARTS = set(os.environ.get("MOE_PARTS", "router,shared,exp").split(","))
AF = mybir.ActivationFunctionType


def emit_body(nc, pools, dram, identity_tile, spc):
    (xp, tp, wstk1, wstk2, hp, yp, g1p, wkp, wkp2, accp, rp, tmpp, xrp,
     ps_h, ps_o, ps_r) = pools
    dma = nc.sync

    def act_silu(out_ap, ps_ap):
        if ACT_COMPAT:
            tmp = tmpp.tile([P, NTOK], f32, tag="tmp")
            nc.scalar.activation(tmp[:], ps_ap, AF.Sigmoid)
            nc.vector.tensor_tensor(out_ap, ps_ap, tmp[:], AluOpType.mult)
        else:
            nc.scalar.activation(out_ap, ps_ap, AF.Silu)

    def act_gelu(out_ap, ps_ap):
        if ACT_COMPAT:
            tmp = tmpp.tile([P, SEG_SLOTS], f32, tag="tmpg")
            nc.scalar.activation(tmp[:], ps_ap, AF.Sigmoid, scale=1.702)
            nc.vector.tensor_tensor(out_ap, ps_ap, tmp[:], AluOpType.mult)
        else:
            nc.scalar.activation(out_ap, ps_ap, AF.Gelu)

    # ---- resident activations (queued first: critical path of seg 0) ----
    xgT = xp.tile([P, KD, spc * SEG_SLOTS], bf16, tag="xgT")
    dma.dma_start(out=xgT[:], in_=dram["xgT"].rearrange("(k p) n -> p k n", p=P))
    xTb = xp.tile([P, KD, NTOK], bf16, tag="xTb")
    dma.dma_start(out=xTb[:], in_=dram["xTb"].rearrange("(k p) n -> p k n", p=P))
    t_sb = tp.tile([P, KD], f32, tag="t_sb")
    dma.dma_start(out=t_sb[:], in_=dram["t_row"].rearrange("(k p) -> p k", p=P))

    w1st = dram["Wstk1"].rearrange("s (k p) h -> p s k h", p=P)
    w2st = dram["Wstk2"].rearrange("s (k p) c -> p s k c", p=P)

    comb = rp.tile([P, NT * E], f32, tag="comb")

    def emit_router():
        if "router" not in PARTS:
            nc.vector.memset(comb[:], 0.125 * OUT_SCALE)
            return
        rw_sb = rp.tile([P, 2 * KD, E], f32, tag="rw")
        dma.dma_start(
            out=rw_sb[:], in_=dram["router_W"].rearrange("(k p) e -> p k e", p=P)
        )
        bias_sb = rp.tile([P, NT * E], f32, tag="bias")
        dma.dma_start(out=bias_sb[:], in_=dram["router_bias_b"][:])

        psR = ps_r.tile([E, NTOK], f32, tag="psR")
        for k in range(KD):
            xr = xrp.tile([P, NTOK], f32, tag="xr")
            dma.dma_start(out=xr[:], in_=dram["x_rt"][k * P:(k + 1) * P, :])
            nc.tensor.matmul(
                psR[:], rw_sb[:, k, :], xr[:],
                start=(k == 0), stop=(k == KD - 1),
            )
        psRt = ps_r.tile([E, 1], f32, tag="psRt")
        for k in range(KD):
            nc.tensor.matmul(
                psRt[:], rw_sb[:, KD + k, :], t_sb[:, k:k + 1],
                start=(k == 0), stop=(k == KD - 1),
            )
        t_logit = rp.tile([E, 1], f32, tag="t_logit")
        nc.vector.tensor_copy(t_logit[:], psRt[:])
        logits_sb = rp.tile([E, NTOK], f32, tag="logits")
        nc.vector.tensor_scalar(
            out=logits_sb[:], in0=psR[:], scalar1=t_logit[:], scalar2=None,
            op0=AluOpType.add,
        )

        psT = ps_r.tile([P, NT * E], f32, tag="psT")
        for t in range(NT):
            nc.tensor.transpose(
                psT[:, t * E:(t + 1) * E],
                logits_sb[:, t * P:(t + 1) * P],
                identity_tile[:E, :E],
            )
        s_sb = rp.tile([P, NT * E], f32, tag="s")
        nc.scalar.activation(s_sb[:], psT[:], AF.Sigmoid)
        sel = rp.tile([P, NT * E], f32, tag="sel")
        nc.vector.tensor_tensor(sel[:], s_sb[:], bias_sb[:], AluOpType.add)

        mx = rp.tile([P, NT * E], f32, tag="mx")
        midx = rp.tile([P, NT * E], mybir.dt.uint32, tag="midx")
        mask = rp.tile([P, NT * E], f32, tag="mask")
        sgated = rp.tile([P, NT * E], f32, tag="sgated")
        denom = rp.tile([P, NT], f32, tag="denom")
        rec = rp.tile([P, NT], f32, tag="rec")
        for t in range(NT):
            sl = slice(t * E, (t + 1) * E)
            nc.vector.max_with_indices(mx[:, sl], midx[:, sl], sel[:, sl])
            nc.vector.tensor_scalar(
                out=mask[:, sl], in0=sel[:, sl],
                scalar1=mx[:, t * E + 1:t * E + 2], scalar2=None,
                op0=AluOpType.is_ge,
            )
            nc.vector.scalar_tensor_tensor(
                out=sgated[:, sl], in0=mask[:, sl], scalar=1.0, in1=s_sb[:, sl],
                op0=AluOpType.mult, op1=AluOpType.mult,
                accum_out=denom[:, t:t + 1],
            )
        nc.vector.tensor_scalar(
            out=denom[:], in0=denom[:], scalar1=1e-9, scalar2=None,
            op0=AluOpType.add,
        )
        nc.vector.reciprocal(out=rec[:], in_=denom[:])
        for t in range(NT):
            sl = slice(t * E, (t + 1) * E)
            nc.vector.tensor_scalar(
                out=comb[:, sl], in0=sgated[:, sl], scalar1=rec[:, t:t + 1],
                scalar2=OUT_SCALE, op0=AluOpType.mult, op1=AluOpType.mult,
            )
        dma.dma_start(out=dram["comb"][:], in_=comb[:])

    # ---- routed expert segments (sparse, expert-pure, ungated) ----
    y_out = dram["y_rt"].rearrange("(s g p) c -> p s g c", p=P, g=ST)
    for seg in range(spc if "exp" in PARTS else 0):
        w1sb = wstk1.tile([P, KD, HE], bf16, tag="w1stk")
        dma.dma_start(out=w1sb[:], in_=w1st[:, seg])
        w2sb = wstk2.tile([P, KH, D], bf16, tag="w2stk")
        dma.dma_start(out=w2sb[:], in_=w2st[:, seg])
        ssl = slice(seg * SEG_SLOTS, (seg + 1) * SEG_SLOTS)
        hT = hp.tile([P, KH, SEG_SLOTS], bf16, tag="hT")
        for j in range(KH):
            ps = ps_h.tile([P, NTOK], f32, tag="psH")
            for k in range(KD):
                nc.tensor.matmul(
                    ps[:, :SEG_SLOTS], w1sb[:, k, j * P:(j + 1) * P],
                    xgT[:, k, ssl],
                    start=(k == 0), stop=(k == KD - 1),
                )
            act_gelu(hT[:, j, :], ps[:, :SEG_SLOTS])
        if seg == 0:
            # router is cheap and off the critical path; run it on the PE
            # while segment 0's second weight stack is still in flight.
            emit_router()
        ysb = yp.tile([P, ST, D], bf16, tag="ysb")
        for st in range(ST):
            for ch in range(2):
                ps2 = ps_o.tile([P, D // 2], f32, tag="psO")
                for k in range(KH):
                    nc.tensor.matmul(
                        ps2[:], hT[:, k, st * P:(st + 1) * P],
                        w2sb[:, k, ch * (D // 2):(ch + 1) * (D // 2)],
                        start=(k == 0), stop=(k == KH - 1),
                    )
                nc.vector.tensor_copy(
                    ysb[:, st, ch * (D // 2):(ch + 1) * (D // 2)], ps2[:]
                )
        dma.dma_start(out=y_out[:, seg], in_=ysb[:])
    if "exp" not in PARTS:
        emit_router()

    # ---- shared expert: g1s = silu(x @ w1); g1s *= (x @ w3); then @ w2 ----
    if "shared" in PARTS:
        g1s = g1p.tile([P, KS, NTOK], bf16, tag="g1s")
        w1h = dram["w1"].rearrange("(k p) x h -> p x k h", p=P)
        w3h = dram["w3"].rearrange("(k p) x h -> p x k h", p=P)
        for wap, is_first in ((w1h, True), (w3h, False)):
            for half in range(2):
                wsh = wkp.tile([P, KD, HS // 2], bf16, tag="wsh")
                dma.dma_start(out=wsh[:], in_=wap[:, half])
                for j in range(KS // 2):
                    jj = half * (KS // 2) + j
                    ps = ps_h.tile([P, NTOK], f32, tag="psH")
                    for k in range(KD):
                        nc.tensor.matmul(
                            ps[:], wsh[:, k, j * P:(j + 1) * P],
                            xTb[:, k, :],
                            start=(k == 0), stop=(k == KD - 1),
                        )
                    if is_first:
                        act_silu(g1s[:, jj, :], ps[:])
                    else:
                        nc.vector.tensor_tensor(
                            g1s[:, jj, :], ps[:], g1s[:, jj, :], AluOpType.mult
                        )
        out_sh = accp.tile([P, NT, D], f32, tag="out_sh")
        w2h = dram["w2"].rearrange("(k p) x c -> p x k c", p=P)
        for ch in range(2):
            w2sh = wkp2.tile([P, KS, D // 2], bf16, tag="w2sh")
            dma.dma_start(out=w2sh[:], in_=w2h[:, ch])
            for t in range(NT):
                ps = ps_o.tile([P, D // 2], f32, tag="psO")
                for k in range(KS):
                    nc.tensor.matmul(
                        ps[:], g1s[:, k, t * P:(t + 1) * P], w2sh[:, k, :],
                        start=(k == 0), stop=(k == KS - 1),
                    )
                nc.vector.tensor_scalar(
                    out=out_sh[:, t, ch * (D // 2):(ch + 1) * (D // 2)],
                    in0=ps[:], scalar1=OUT_SCALE, scalar2=None,
                    op0=AluOpType.mult,
                )
        dma.dma_start(
            out=dram["out_sh"].rearrange("(g p) c -> p g c", p=P),
            in_=out_sh[:],
        )


def build_nc(reps=1, spc=3):
    nc = bacc.Bacc(None, target_bir_lowering=False, debug=False)
    sg = spc * SEG_SLOTS
    dram = {
        "xTb": nc.dram_tensor("xTb", [D, NTOK], bf16, kind="ExternalInput").ap(),
        "x_rt": nc.dram_tensor("x_rt", [D, NTOK], f32, kind="ExternalInput").ap(),
        "t_row": nc.dram_tensor("t_row", [D], f32, kind="ExternalInput").ap(),
        "router_W": nc.dram_tensor(
            "router_W", [2 * D, E], f32, kind="ExternalInput").ap(),
        "router_bias_b": nc.dram_tensor(
            "router_bias_b", [P, NT * E], f32, kind="ExternalInput").ap(),
        "w1": nc.dram_tensor("w1", [D, 2, HS // 2], bf16, kind="ExternalInput").ap(),
        "w3": nc.dram_tensor("w3", [D, 2, HS // 2], bf16, kind="ExternalInput").ap(),
        "w2": nc.dram_tensor("w2", [HS, 2, D // 2], bf16, kind="ExternalInput").ap(),
        "Wstk1": nc.dram_tensor(
            "Wstk1", [spc, D, HE], bf16, kind="ExternalInput").ap(),
        "Wstk2": nc.dram_tensor(
            "Wstk2", [spc, HE, D], bf16, kind="ExternalInput").ap(),
        "xgT": nc.dram_tensor("xgT", [D, sg], bf16, kind="ExternalInput").ap(),
        "out_sh": nc.dram_tensor(
            "out_sh", [NTOK, D], f32, kind="ExternalOutput").ap(),
        "y_rt": nc.dram_tensor("y_rt", [sg, D], bf16, kind="ExternalOutput").ap(),
        "comb": nc.dram_tensor(
            "comb", [P, NT * E], f32, kind="ExternalOutput").ap(),
    }
    with tile.TileContext(nc) as tc:
        with ExitStack() as ctx:
            const = ctx.enter_context(tc.tile_pool(name="const", bufs=1))
            xp = ctx.enter_context(tc.tile_pool(name="xp", bufs=1))
            tp = ctx.enter_context(tc.tile_pool(name="tp", bufs=1))
            wstk1 = ctx.enter_context(tc.tile_pool(name="wstk1", bufs=2))
            wstk2 = ctx.enter_context(tc.tile_pool(name="wstk2", bufs=2))
            hp = ctx.enter_context(tc.tile_pool(name="hp", bufs=1))
            yp = ctx.enter_context(tc.tile_pool(name="yp", bufs=1))
            g1p = ctx.enter_context(tc.tile_pool(name="g1p", bufs=1))
            wkp = ctx.enter_context(tc.tile_pool(name="wkp", bufs=2))
            wkp2 = ctx.enter_context(tc.tile_pool(name="wkp2", bufs=2))
            accp = ctx.enter_context(tc.tile_pool(name="accp", bufs=1))
            rp = ctx.enter_context(tc.tile_pool(name="rp", bufs=1))
            tmpp = (ctx.enter_context(tc.tile_pool(name="tmpp", bufs=2))
                    if ACT_COMPAT else None)
            xrp = ctx.enter_context(tc.tile_pool(name="xrp", bufs=2))
            ps_h = ctx.enter_context(tc.tile_pool(name="ps_h", bufs=2, space="PSUM"))
            ps_o = ctx.enter_context(tc.tile_pool(name="ps_o", bufs=3, space="PSUM"))
            ps_r = ctx.enter_context(tc.tile_pool(name="ps_r", bufs=1, space="PSUM"))
            pools = (xp, tp, wstk1, wstk2, hp, yp, g1p, wkp, wkp2, accp, rp,
                     tmpp, xrp, ps_h, ps_o, ps_r)

            identity_tile = const.tile([P, P], f32, tag="ident")
            make_identity(nc, identity_tile[:])

            if reps == 1:
                emit_body(nc, pools, dram, identity_tile, spc)
            else:
                with tc.For_i(0, reps, 1):
                    emit_body(nc, pools, dram, identity_tile, spc)
    nc.compile()
    return nc


def _bf16():
    import ml_dtypes
    return np.dtype(ml_dtypes.bfloat16)


def _route_host(x, t_emb, router_W, router_bias):
    """Replicates the reference router in numpy (f32) for layout only."""
    N = B * T
    xf = np.asarray(x, np.float32).reshape(N, D)
    rw = np.asarray(router_W, np.float32)
    tl = np.asarray(t_emb, np.float32) @ rw[D:]            # [B, E]
    logits = xf @ rw[:D] + tl[np.arange(N) // T]
    s = 1.0 / (1.0 + np.exp(-logits))
    sel = s + np.asarray(router_bias, np.float32)
    order = np.argsort(-sel, axis=1, kind="stable")[:, :TOPK]  # [N, K]
    return xf, order


def _prepare(x, t_emb, router_W, router_bias, w1, w3, w2, W1e, W2e):
    """Host shard step: routing layout + per-core in_maps + scatter metadata."""
    mmnp = _bf16()
    N = B * T
    xf, topk = _route_host(x, t_emb, router_W, router_bias)

    # expert-sorted slot list, groups padded to SEG_SLOTS multiples
    segs = []  # (expert, token_ids)
    for e in range(E):
        toks = np.where((topk == e).any(axis=1))[0]
        for i in range(0, max(len(toks), 1), SEG_SLOTS):
            segs.append((e, toks[i:i + SEG_SLOTS]))
    spc = -(-len(segs) // NCORES)
    while len(segs) < NCORES * spc:
        segs.append((0, np.empty(0, np.int64)))

    bias_b = np.ascontiguousarray(
        np.tile(np.asarray(router_bias, np.float32)[None, :], (P, NT))
    )
    shared = {
        "router_W": np.ascontiguousarray(router_W, np.float32),
        "router_bias_b": bias_b,
        "w1": np.ascontiguousarray(np.asarray(w1, np.float32)
                                   .reshape(D, 2, HS // 2).astype(mmnp)),
        "w3": np.ascontiguousarray(np.asarray(w3, np.float32)
                                   .reshape(D, 2, HS // 2).astype(mmnp)),
        "w2": np.ascontiguousarray(np.asarray(w2, np.float32)
                                   .reshape(HS, 2, D // 2).astype(mmnp)),
    }
    W1b = np.asarray(W1e, np.float32).astype(mmnp)
    W2b = np.asarray(W2e, np.float32).astype(mmnp)

    in_maps = []
    slot_tok_all, slot_exp_all, slot_valid_all = [], [], []
    for c in range(NCORES):
        chunk = xf[c * NTOK:(c + 1) * NTOK]
        bidx = (c * NTOK) // T
        csegs = segs[c * spc:(c + 1) * spc]
        stoks = np.zeros(spc * SEG_SLOTS, np.int64)
        svalid = np.zeros(spc * SEG_SLOTS, bool)
        sexp = np.zeros(spc * SEG_SLOTS, np.int64)
        for i, (e, toks) in enumerate(csegs):
            stoks[i * SEG_SLOTS:i * SEG_SLOTS + len(toks)] = toks
            svalid[i * SEG_SLOTS:i * SEG_SLOTS + len(toks)] = True
            sexp[i * SEG_SLOTS:(i + 1) * SEG_SLOTS] = e
        slot_tok_all.append(stoks)
        slot_exp_all.append(sexp)
        slot_valid_all.append(svalid)
        xT = np.ascontiguousarray(chunk.T)  # [D, 512] f32
        in_maps.append({
            "xTb": xT.astype(mmnp),
            "x_rt": xT,
            "t_row": np.ascontiguousarray(np.asarray(t_emb, np.float32)[bidx]),
            "Wstk1": np.ascontiguousarray(W1b[[e for e, _ in csegs]]),
            "Wstk2": np.ascontiguousarray(W2b[[e for e, _ in csegs]]),
            "xgT": np.ascontiguousarray(xf[stoks].T.astype(mmnp)),
            **shared,
        })
    meta = {
        "spc": spc,
        "slot_tok": np.concatenate(slot_tok_all),
        "slot_exp": np.concatenate(slot_exp_all),
        "slot_valid": np.concatenate(slot_valid_all),
    }
    return in_maps, meta


_NC_CACHE = {}
_LAST_SPC = 3


def get_nc(reps=1):
    key = (reps, _LAST_SPC)
    if key not in _NC_CACHE:
        _NC_CACHE[key] = build_nc(reps, _LAST_SPC)
    return _NC_CACHE[key]


def make_in_maps(x, t_emb, router_W, router_bias, w1, w3, w2, W1e, W2e):
    global _LAST_SPC
    in_maps, meta = _prepare(x, t_emb, router_W, router_bias, w1, w3, w2,
                             W1e, W2e)
    _LAST_SPC = meta["spc"]
    return in_maps


def kernel(x, t_emb, router_W, router_bias, w1, w3, w2, W1e, W2e):
    global _LAST_SPC
    in_maps, meta = _prepare(x, t_emb, router_W, router_bias, w1, w3, w2,
                             W1e, W2e)
    _LAST_SPC = meta["spc"]
    nc = get_nc(reps=1)
    r = run_bass_kernel_spmd(nc, in_maps, list(range(NCORES)), trace=False)

    # ---- host unshard: scatter-add gated expert outputs onto shared ----
    N = B * T
    comb_full = np.concatenate([
        np.asarray(r.results[c]["comb"], np.float32)
        .reshape(P, NT, E).transpose(1, 0, 2).reshape(NTOK, E)
        for c in range(NCORES)
    ], axis=0)                                             # [N, E], device gates
    out = np.concatenate([
        np.asarray(r.results[c]["out_sh"], np.float32) for c in range(NCORES)
    ], axis=0)                                             # [N, D]
    y = np.concatenate([
        np.asarray(r.results[c]["y_rt"]).astype(np.float32)
        for c in range(NCORES)
    ], axis=0)                                             # [nslots, D]

    valid = meta["slot_valid"]
    tok = meta["slot_tok"][valid]
    gates = comb_full[tok, meta["slot_exp"][valid]]        # device-computed
    yv = y[valid]
    # each token has at most TOPK slots; split into rounds of unique tokens
    order = np.argsort(tok, kind="stable")
    st = tok[order]
    rank = np.arange(len(st)) - np.maximum.accumulate(
        np.where(np.r_[True, st[1:] != st[:-1]], np.arange(len(st)), 0))
    occ = np.empty(len(tok), np.int64)
    occ[order] = rank
    for rd in range(TOPK):
        m = occ == rd
        out[tok[m]] += gates[m, None] * yv[m]
    return out.reshape(B, T, D).astype(np.float32)
